# revision 8
# baseline (speedup 1.0000x reference)
"""MoE dense all-experts (GPT-OSS Experts forward) on 8 Trainium2 NeuronCores.

Expert-parallel sharding: core e holds expert e's weights and computes its
weighted contribution

    partial_e[t, h] = w[t, e] * ((up + 1) * silu(1.702 * gate) @ down_e.T + db_e)

with [gate | up] = hs @ gup_e + bias (the host de-interleaves gup's even/odd
columns so gate/up become contiguous halves).

All matmul operands are bf16 (fp32 PSUM accumulation): same 1 cycle/row PE
rate as float32r, but LDWEIGHTS gets the compiler's fast-weight-load path
(disabled for fp32 operands) and every DMA byte count halves. Each 128x128
stationary tile is reused for two 512-row moving matmuls (paired PSUM banks,
1024-token chunks), halving LDWEIGHTS count. End-to-end relative error vs the
fp32 reference is ~4e-3, well inside the 2e-2 gate.

Tokens are processed in four 1024-token chunks. The first three chunks'
partials are summed across cores with a bf16 ReduceScatter that overlaps the
next chunk's compute; the last chunk ships per-core partials (otail) that the
host sums in fp32, so the device-side tail ends with compute, not a
collective. Host DMA layouts are pre-tiled ([128, kc*...]) so every transfer
has 2KB row lines and the j=0 gate chain lands first, letting the PE start
~2us in.
"""
import sys
if '/opt/trn_rl_repo' not in sys.path:
    sys.path.insert(0, '/opt/trn_rl_repo')
import numpy as np
import ml_dtypes

BF16 = ml_dtypes.bfloat16

E, H, I, T = 8, 1024, 1024, 4096
N_CORES = 8
KC = H // 128           # contraction tiles (H == I == 1024)
NJ = I // 128            # gate/up 128-row f-tiles per half
CHUNK = 1024
NCH = T // CHUNK         # 4 chunks; first NCH-1 are ReduceScattered
HB = 512                 # psum bank width in fp32 elements

_CACHE = {}


def _dedup_ldweights(m):
    """Drop InstLdweights that reload the stationary already in the PE array.

    bass emits an InstLdweights before every InstMatmult; consecutive matmuls
    here deliberately share one 128x128 stationary, so every second load is
    redundant (the PE array retains weights across matmuls). Identity is the
    full physical access pattern repr; any non-matmul PE instruction resets
    tracking. The removed load's sync deps are merged into the surviving one.
    """
    removed = 0
    for blk in m.functions[0].blocks:
        last = None          # (signature, surviving InstLdweights)
        for inst in list(blk.instructions):
            tn = type(inst).__name__
            if tn == 'InstLdweights':
                sig = repr(inst.ins[0]) + repr(inst.is_transpose) + repr(inst.perf_mode)
                if last is not None and last[0] == sig:
                    last[1].merge_dependencies_from(inst)
                    blk.instructions.remove(inst)
                    removed += 1
                else:
                    last = (sig, inst)
            elif tn != 'InstMatmult' and getattr(inst, 'engine', None) is not None \
                    and 'PE' in str(inst.engine):
                last = None
    return removed


def _build():
    import concourse.bacc as bacc
    import concourse.tile as tile
    import concourse.mybir as mybir
    f32 = mybir.dt.float32
    bf16 = mybir.dt.bfloat16
    AF = mybir.ActivationFunctionType
    ALU = mybir.AluOpType

    nc = bacc.Bacc("TRN2", target_bir_lowering=False, debug=False,
                   enable_asserts=False, num_devices=N_CORES)
    # host-pretiled layouts: partition dim first, 2KB-line-friendly columns
    hsT = nc.dram_tensor("hsT", [128, KC * T], bf16, kind="ExternalInput").ap()
    gup = nc.dram_tensor("gup", [128, 2 * NJ * KC * 128], bf16, kind="ExternalInput").ap()
    gb = nc.dram_tensor("gb", [128, NJ], f32, kind="ExternalInput").ap()
    ub = nc.dram_tensor("ub", [128, NJ], f32, kind="ExternalInput").ap()
    dwT = nc.dram_tensor("dwT", [128, KC * H], bf16, kind="ExternalInput").ap()
    db = nc.dram_tensor("db", [1, H], f32, kind="ExternalInput").ap()
    wt = nc.dram_tensor("wt", [128, T // 128], f32, kind="ExternalInput").ap()
    osh = nc.dram_tensor("osh", [(NCH - 1) * 128, H], bf16, kind="ExternalOutput").ap()
    otail = nc.dram_tensor("otail", [CHUNK, H], bf16, kind="ExternalOutput").ap()

    with tile.TileContext(nc) as tc_:
        with tc_.tile_pool(name="wpool", bufs=1) as wpool, \
             tc_.tile_pool(name="hpool", bufs=2) as hpool, \
             tc_.tile_pool(name="apool", bufs=2) as apool, \
             tc_.tile_pool(name="spool", bufs=8) as spool, \
             tc_.tile_pool(name="opool", bufs=3) as opool, \
             tc_.tile_pool(name="bpool", bufs=2) as bpool, \
             tc_.tile_pool(name="dpool", bufs=3, space="DRAM") as dpool, \
             tc_.tile_pool(name="ps1", bufs=2, space="PSUM") as ps1, \
             tc_.tile_pool(name="ps2", bufs=2, space="PSUM") as ps2:

            gup_r = wpool.tile([128, 2 * NJ * KC * 128], bf16)
            dwT_r = wpool.tile([128, KC * H], bf16)
            gb_r = wpool.tile([128, NJ], f32)
            ub_r = wpool.tile([128, NJ], f32)
            db_f = wpool.tile([1, H], f32)
            db_bc = wpool.tile([128, H], f32)
            w_r = wpool.tile([128, T // 128], f32)

            # DMA order == consumption order: tiny bias/route tensors, the
            # j=0 gate tiles, chunk-0 hs (kc-major halves, paced with the j=0
            # accumulation chain), remaining gate then up tiles, down weights,
            # chunk-1 hs prefetch.
            nc.sync.dma_start(gb_r[:], gb[:])
            nc.sync.dma_start(ub_r[:], ub[:])
            nc.sync.dma_start(db_f[:], db[:])
            nc.sync.dma_start(w_r[:], wt[:])
            JW = KC * 128        # columns per f-tile block in gup layout
            nc.sync.dma_start(gup_r[:, 0:JW], gup[:, 0:JW])
            hs0 = hpool.tile([128, KC * CHUNK], bf16, tag="hs")
            for kc in range(KC):
                for hf in range(2):
                    nc.sync.dma_start(
                        hs0[:, kc * CHUNK + hf * HB: kc * CHUNK + (hf + 1) * HB],
                        hsT[:, kc * T + hf * HB: kc * T + (hf + 1) * HB])
            for j2 in range(1, 2 * NJ):
                nc.sync.dma_start(gup_r[:, j2 * JW:(j2 + 1) * JW],
                                  gup[:, j2 * JW:(j2 + 1) * JW])
            for ic in range(KC):
                nc.sync.dma_start(dwT_r[:, ic * H:(ic + 1) * H],
                                  dwT[:, ic * H:(ic + 1) * H])
            hs1 = hpool.tile([128, KC * CHUNK], bf16, tag="hs")
            nc.sync.dma_start(
                hs1[:].rearrange("p (kc t) -> p kc t", t=CHUNK),
                hsT[:].rearrange("p (kc t) -> p kc t", t=T)[:, :, CHUNK:2 * CHUNK])
            nc.gpsimd.partition_broadcast(db_bc[:], db_f[:])

            for c in range(NCH):
                t_off = c * CHUNK
                if c == 0:
                    hs_r = hs0
                elif c == 1:
                    hs_r = hs1
                else:
                    hs_r = hpool.tile([128, KC * CHUNK], bf16, tag="hs")
                    nc.sync.dma_start(
                        hs_r[:].rearrange("p (kc t) -> p kc t", t=CHUNK),
                        hsT[:].rearrange("p (kc t) -> p kc t", t=T)[:, :, t_off:t_off + CHUNK])

                act_r = apool.tile([128, NJ * CHUNK], bf16, tag="act")
                s2s = []
                for j in range(NJ):     # gate pass
                    pg = ps1.tile([128, CHUNK], f32, tag="p1")
                    for hf in range(2):  # same-bank runs of KC matmuls
                        for kc in range(KC):
                            st = gup_r[:, j * JW + kc * 128: j * JW + (kc + 1) * 128]
                            nc.tensor.matmul(pg[:, hf * HB:(hf + 1) * HB], st,
                                             hs_r[:, kc * CHUNK + hf * HB: kc * CHUNK + (hf + 1) * HB],
                                             start=(kc == 0), stop=(kc == KC - 1))
                    s2 = spool.tile([128, CHUNK], bf16, tag="s2")
                    nc.scalar.activation(s2[:], pg[:], AF.Silu,
                                         bias=gb_r[:, j:j + 1], scale=1.702)
                    s2s.append(s2)
                for j in range(NJ):     # up pass: act = (up + ub + 1) * silu_out
                    pu = ps1.tile([128, CHUNK], f32, tag="p1")
                    for hf in range(2):
                        for kc in range(KC):
                            st = gup_r[:, (NJ + j) * JW + kc * 128:(NJ + j) * JW + (kc + 1) * 128]
                            nc.tensor.matmul(pu[:, hf * HB:(hf + 1) * HB], st,
                                             hs_r[:, kc * CHUNK + hf * HB: kc * CHUNK + (hf + 1) * HB],
                                             start=(kc == 0), stop=(kc == KC - 1))
                    nc.vector.scalar_tensor_tensor(act_r[:, j * CHUNK:(j + 1) * CHUNK],
                                                   pu[:], ub_r[:, j:j + 1], s2s[j][:],
                                                   op0=ALU.add, op1=ALU.mult)

                last = (c == NCH - 1)
                if not last:
                    bin_ = dpool.tile([CHUNK, H], bf16, tag="bi")
                    bout = dpool.tile([128, H], bf16, tag="bo")
                for tt in range(CHUNK // 128):
                    gt = (t_off // 128) + tt
                    wcol = w_r[:, gt:gt + 1]
                    ot = opool.tile([128, H], bf16, tag="ot")
                    p2 = ps2.tile([128, H], f32, tag="p2")
                    for hh in range(2):  # same-bank runs of KC matmuls
                        for ic in range(KC):
                            st = act_r[:, ic * CHUNK + tt * 128: ic * CHUNK + (tt + 1) * 128]
                            nc.tensor.matmul(p2[:, hh * HB:(hh + 1) * HB], st,
                                             dwT_r[:, ic * H + hh * HB: ic * H + (hh + 1) * HB],
                                             start=(ic == 0), stop=(ic == KC - 1))
                    dbw = bpool.tile([128, H], f32, tag="dbw")
                    nc.vector.tensor_scalar_mul(dbw[:], db_bc[:], wcol)
                    nc.vector.scalar_tensor_tensor(ot[:], p2[:], wcol, dbw[:],
                                                   op0=ALU.mult, op1=ALU.add)
                    if last:
                        # final chunk: ship per-core partials; the host sums
                        # them in fp32 so the device tail ends with compute
                        nc.sync.dma_start(otail[tt * 128:(tt + 1) * 128, :], ot[:])
                    else:
                        nc.sync.dma_start(bin_[tt * 128:(tt + 1) * 128, :], ot[:])
                if not last:
                    nc.gpsimd.collective_compute(
                        "ReduceScatter", ALU.add,
                        replica_groups=[list(range(N_CORES))],
                        ins=[bin_[:].opt()], outs=[bout[:].opt()])
                    nc.sync.dma_start(osh[c * 128:(c + 1) * 128, :], bout[:])
    _dedup_ldweights(nc.m)
    nc.compile()
    return nc


def _get_nc():
    if 'nc' not in _CACHE:
        _CACHE['nc'] = _build()
    return _CACHE['nc']


def _make_in_maps(hidden_states, routing_weights, gate_up_proj, gate_up_proj_bias,
                  down_proj, down_proj_bias):
    hs = np.asarray(hidden_states, dtype=np.float32)
    rw = np.asarray(routing_weights, dtype=np.float32)
    gupw = np.asarray(gate_up_proj, dtype=np.float32)
    gupb = np.asarray(gate_up_proj_bias, dtype=np.float32)
    dw = np.asarray(down_proj, dtype=np.float32)
    dbias = np.asarray(down_proj_bias, dtype=np.float32)
    # hsT layout [128, kc*T]: element (h=kc*128+p, t) -> [p, kc*T + t]
    hsT = np.ascontiguousarray(
        hs.T.reshape(KC, 128, T).transpose(1, 0, 2).reshape(128, KC * T).astype(BF16))
    wt_all = np.ascontiguousarray(rw.reshape(T // 128, 128, E).transpose(1, 0, 2))
    in_maps = []
    for e in range(N_CORES):
        g = gupw[e]
        # de-interleave: [gate | up], then tile [128, f_tile*(KC*128) + kc*128 + q]
        gup_de = np.concatenate([g[:, 0::2], g[:, 1::2]], axis=1)  # [H, 2I]
        gup_t = (gup_de.reshape(KC, 128, 2 * NJ, 128).transpose(1, 2, 0, 3)
                 .reshape(128, 2 * NJ * KC * 128).astype(BF16))
        # dwT layout [128, ic*H + h]: element (i=ic*128+p, h) of dw[e].T/1.702
        dwt = ((dw[e].T / np.float32(1.702)).reshape(KC, 128, H)
               .transpose(1, 0, 2).reshape(128, KC * H).astype(BF16))
        in_maps.append({
            "hsT": hsT,
            "gup": np.ascontiguousarray(gup_t),
            # silu(1.702*(x + b)) = silu(1.702*x + 1.702*b); the 1/1.702 glu
            # scale is folded into dwT above.
            "gb": np.ascontiguousarray((1.702 * gupb[e, 0::2]).reshape(NJ, 128).T),
            "ub": np.ascontiguousarray((gupb[e, 1::2] + 1.0).reshape(NJ, 128).T),
            "dwT": np.ascontiguousarray(dwt),
            "db": np.ascontiguousarray(dbias[e][None, :]),
            "wt": np.ascontiguousarray(wt_all[:, :, e]),
        })
    return in_maps


def _assemble(results):
    out = np.empty((T, H), dtype=np.float32)
    for c in range(NCH - 1):
        for r in range(N_CORES):
            out[c * CHUNK + r * 128: c * CHUNK + (r + 1) * 128, :] = \
                results[r]["osh"][c * 128:(c + 1) * 128, :].astype(np.float32)
    tail = np.zeros((CHUNK, H), dtype=np.float32)
    for r in range(N_CORES):
        tail += results[r]["otail"].astype(np.float32)
    out[(NCH - 1) * CHUNK:, :] = tail
    return out


def kernel(hidden_states, routing_weights, gate_up_proj, gate_up_proj_bias,
           down_proj, down_proj_bias):
    from concourse import bass_utils
    in_maps = _make_in_maps(hidden_states, routing_weights, gate_up_proj,
                            gate_up_proj_bias, down_proj, down_proj_bias)
    nc = _get_nc()
    try:
        res = bass_utils.run_bass_kernel_spmd(nc, in_maps, core_ids=list(range(N_CORES)))
    except Exception:
        # One retry in case a previous process left a core wedged.
        res = bass_utils.run_bass_kernel_spmd(nc, in_maps, core_ids=list(range(N_CORES)))
    return _assemble(res.results)


# revision 12
# speedup vs baseline: 1.0138x; 1.0138x over previous
"""MoE dense all-experts (GPT-OSS Experts forward) on 8 Trainium2 NeuronCores.

Expert-parallel sharding: core e holds expert e's weights and computes its
weighted contribution

    partial_e[t, h] = w[t, e] * ((up + 1) * silu(1.702 * gate) @ down_e.T + db_e)

with [gate | up] = hs @ gup_e + bias (the host de-interleaves gup's even/odd
columns so gate/up become contiguous halves).

All matmul operands are bf16 (fp32 PSUM accumulation): same 1 cycle/row PE
rate as float32r, but LDWEIGHTS gets the compiler's fast-weight-load path
(disabled for fp32 operands) and every DMA byte count halves. Each 128x128
stationary tile is reused for two 512-row moving matmuls (paired PSUM banks,
1024-token chunks), halving LDWEIGHTS count. End-to-end relative error vs the
fp32 reference is ~4e-3, well inside the 2e-2 gate.

Tokens are processed in four 1024-token chunks. The first three chunks'
partials are summed across cores with a bf16 ReduceScatter that overlaps the
next chunk's compute; the last chunk ships per-core partials (otail) that the
host sums in fp32, so the device-side tail ends with compute, not a
collective. Host DMA layouts are pre-tiled ([128, kc*...]) so every transfer
has 2KB row lines and the j=0 gate chain lands first, letting the PE start
~2us in.
"""
import sys
if '/opt/trn_rl_repo' not in sys.path:
    sys.path.insert(0, '/opt/trn_rl_repo')
import numpy as np
import ml_dtypes

BF16 = ml_dtypes.bfloat16

E, H, I, T = 8, 1024, 1024, 4096
N_CORES = 8
KC = H // 128           # contraction tiles (H == I == 1024)
NJ = I // 128            # gate/up 128-row f-tiles per half
CHUNK = 1024
NCH = T // CHUNK         # 4 chunks; first NCH-1 are ReduceScattered
HB = 512                 # psum bank width in fp32 elements

_CACHE = {}


def _dedup_ldweights(m):
    """Drop InstLdweights that reload the stationary already in the PE array.

    bass emits an InstLdweights before every InstMatmult; consecutive matmuls
    here deliberately share one 128x128 stationary, so every second load is
    redundant (the PE array retains weights across matmuls). Identity is the
    full physical access pattern repr; any non-matmul PE instruction resets
    tracking. The removed load's sync deps are merged into the surviving one.
    """
    removed = 0
    for blk in m.functions[0].blocks:
        last = None          # (signature, surviving InstLdweights)
        for inst in list(blk.instructions):
            tn = type(inst).__name__
            if tn == 'InstLdweights':
                sig = repr(inst.ins[0]) + repr(inst.is_transpose) + repr(inst.perf_mode)
                if last is not None and last[0] == sig:
                    last[1].merge_dependencies_from(inst)
                    blk.instructions.remove(inst)
                    removed += 1
                else:
                    last = (sig, inst)
            elif tn != 'InstMatmult' and getattr(inst, 'engine', None) is not None \
                    and 'PE' in str(inst.engine):
                last = None
    return removed


def _build():
    import concourse.bacc as bacc
    import concourse.tile as tile
    import concourse.mybir as mybir
    f32 = mybir.dt.float32
    bf16 = mybir.dt.bfloat16
    AF = mybir.ActivationFunctionType
    ALU = mybir.AluOpType

    nc = bacc.Bacc("TRN2", target_bir_lowering=False, debug=False,
                   enable_asserts=False, num_devices=N_CORES)
    # host-pretiled layouts: partition dim first, 2KB-line-friendly columns
    hsT = nc.dram_tensor("hsT", [128, KC * T], bf16, kind="ExternalInput").ap()
    gup = nc.dram_tensor("gup", [128, 2 * NJ * KC * 128], bf16, kind="ExternalInput").ap()
    gb = nc.dram_tensor("gb", [128, NJ], f32, kind="ExternalInput").ap()
    ub = nc.dram_tensor("ub", [128, NJ], f32, kind="ExternalInput").ap()
    dwT = nc.dram_tensor("dwT", [128, KC * H], bf16, kind="ExternalInput").ap()
    db = nc.dram_tensor("db", [1, H], f32, kind="ExternalInput").ap()
    wt = nc.dram_tensor("wt", [128, T // 128], f32, kind="ExternalInput").ap()
    osh = nc.dram_tensor("osh", [(NCH - 1) * 128, H], bf16, kind="ExternalOutput").ap()
    otail = nc.dram_tensor("otail", [CHUNK, H], bf16, kind="ExternalOutput").ap()

    with tile.TileContext(nc) as tc_:
        with tc_.tile_pool(name="wpool", bufs=1) as wpool, \
             tc_.tile_pool(name="hpool", bufs=2) as hpool, \
             tc_.tile_pool(name="apool", bufs=2) as apool, \
             tc_.tile_pool(name="spool", bufs=8) as spool, \
             tc_.tile_pool(name="opool", bufs=3) as opool, \
             tc_.tile_pool(name="bpool", bufs=2) as bpool, \
             tc_.tile_pool(name="dpool", bufs=3, space="DRAM") as dpool, \
             tc_.tile_pool(name="ps1", bufs=2, space="PSUM") as ps1, \
             tc_.tile_pool(name="ps2", bufs=2, space="PSUM") as ps2:

            gup_r = wpool.tile([128, 2 * NJ * KC * 128], bf16)
            dwT_r = wpool.tile([128, KC * H], bf16)
            gb_r = wpool.tile([128, NJ], f32)
            ub_r = wpool.tile([128, NJ], f32)
            db_f = wpool.tile([1, H], f32)
            db_bc = wpool.tile([128, H], f32)
            w_r = wpool.tile([128, T // 128], f32)

            # DMA order == consumption order: tiny bias/route tensors, the
            # j=0 gate tiles, chunk-0 hs (kc-major halves, paced with the j=0
            # accumulation chain), remaining gate then up tiles, down weights,
            # chunk-1 hs prefetch.
            nc.sync.dma_start(gb_r[:], gb[:])
            nc.sync.dma_start(ub_r[:], ub[:])
            nc.sync.dma_start(db_f[:], db[:])
            nc.sync.dma_start(w_r[:], wt[:])
            JW = KC * 128        # columns per f-tile block in gup layout
            # gate-pass warmup order == consumption order: j0 weights, the
            # kc-major hs halves j0 reads, j1 weights, the second halves, then
            # the remaining gate and up tiles (DMA stays ahead of the PE).
            nc.sync.dma_start(gup_r[:, 0:JW], gup[:, 0:JW])
            hs0 = hpool.tile([128, KC * CHUNK], bf16, tag="hs")
            for kc in range(KC):
                nc.sync.dma_start(hs0[:, kc * CHUNK:(kc + 1) * CHUNK],
                                  hsT[:, kc * T: kc * T + CHUNK])
                if kc < 2:  # j1/j2 weights land before j0's chain finishes
                    nc.sync.dma_start(gup_r[:, (kc + 1) * JW:(kc + 2) * JW],
                                      gup[:, (kc + 1) * JW:(kc + 2) * JW])
            for j2 in range(3, 2 * NJ):
                nc.sync.dma_start(gup_r[:, j2 * JW:(j2 + 1) * JW],
                                  gup[:, j2 * JW:(j2 + 1) * JW])
            for ic in range(KC):
                nc.sync.dma_start(dwT_r[:, ic * H:(ic + 1) * H],
                                  dwT[:, ic * H:(ic + 1) * H])
            hs1 = hpool.tile([128, KC * CHUNK], bf16, tag="hs")
            nc.sync.dma_start(
                hs1[:].rearrange("p (kc t) -> p kc t", t=CHUNK),
                hsT[:].rearrange("p (kc t) -> p kc t", t=T)[:, :, CHUNK:2 * CHUNK])
            nc.gpsimd.partition_broadcast(db_bc[:], db_f[:])

            for c in range(NCH):
                t_off = c * CHUNK
                if c == 0:
                    hs_r = hs0
                elif c == 1:
                    hs_r = hs1
                else:
                    hs_r = hpool.tile([128, KC * CHUNK], bf16, tag="hs")
                    nc.sync.dma_start(
                        hs_r[:].rearrange("p (kc t) -> p kc t", t=CHUNK),
                        hsT[:].rearrange("p (kc t) -> p kc t", t=T)[:, :, t_off:t_off + CHUNK])

                act_r = apool.tile([128, NJ * CHUNK], bf16, tag="act")
                s2s = []
                for j in range(NJ):     # gate pass
                    pg = ps1.tile([128, CHUNK], f32, tag="p1")
                    for kc in range(KC):  # one stationary load per (j, kc) pair
                        st = gup_r[:, j * JW + kc * 128: j * JW + (kc + 1) * 128]
                        nc.tensor.matmul(pg[:, 0:HB], st,
                                         hs_r[:, kc * CHUNK: kc * CHUNK + HB],
                                         start=(kc == 0), stop=(kc == KC - 1))
                        nc.tensor.matmul(pg[:, HB:CHUNK], st,
                                         hs_r[:, kc * CHUNK + HB:(kc + 1) * CHUNK],
                                         start=(kc == 0), stop=(kc == KC - 1))
                    s2 = spool.tile([128, CHUNK], bf16, tag="s2")
                    nc.scalar.activation(s2[:], pg[:], AF.Silu,
                                         bias=gb_r[:, j:j + 1], scale=1.702)
                    s2s.append(s2)
                for j in range(NJ):     # up pass: act = (up + ub + 1) * silu_out
                    pu = ps1.tile([128, CHUNK], f32, tag="p1")
                    for kc in range(KC):
                        st = gup_r[:, (NJ + j) * JW + kc * 128:(NJ + j) * JW + (kc + 1) * 128]
                        nc.tensor.matmul(pu[:, 0:HB], st,
                                         hs_r[:, kc * CHUNK: kc * CHUNK + HB],
                                         start=(kc == 0), stop=(kc == KC - 1))
                        nc.tensor.matmul(pu[:, HB:CHUNK], st,
                                         hs_r[:, kc * CHUNK + HB:(kc + 1) * CHUNK],
                                         start=(kc == 0), stop=(kc == KC - 1))
                    nc.vector.scalar_tensor_tensor(act_r[:, j * CHUNK:(j + 1) * CHUNK],
                                                   pu[:], ub_r[:, j:j + 1], s2s[j][:],
                                                   op0=ALU.add, op1=ALU.mult)

                last = (c == NCH - 1)
                if not last:
                    bin_ = dpool.tile([CHUNK, H], bf16, tag="bi")
                    bout = dpool.tile([128, H], bf16, tag="bo")
                for tt in range(CHUNK // 128):
                    gt = (t_off // 128) + tt
                    wcol = w_r[:, gt:gt + 1]
                    ot = opool.tile([128, H], bf16, tag="ot")
                    p2 = ps2.tile([128, H], f32, tag="p2")
                    for ic in range(KC):  # one stationary load per (tt, ic) pair
                        st = act_r[:, ic * CHUNK + tt * 128: ic * CHUNK + (tt + 1) * 128]
                        nc.tensor.matmul(p2[:, 0:HB], st, dwT_r[:, ic * H: ic * H + HB],
                                         start=(ic == 0), stop=(ic == KC - 1))
                        nc.tensor.matmul(p2[:, HB:H], st, dwT_r[:, ic * H + HB:(ic + 1) * H],
                                         start=(ic == 0), stop=(ic == KC - 1))
                    dbw = bpool.tile([128, H], f32, tag="dbw")
                    nc.vector.tensor_scalar_mul(dbw[:], db_bc[:], wcol)
                    nc.vector.scalar_tensor_tensor(ot[:], p2[:], wcol, dbw[:],
                                                   op0=ALU.mult, op1=ALU.add)
                    if last:
                        # final chunk: ship per-core partials; the host sums
                        # them in fp32 so the device tail ends with compute
                        nc.sync.dma_start(otail[tt * 128:(tt + 1) * 128, :], ot[:])
                    else:
                        nc.sync.dma_start(bin_[tt * 128:(tt + 1) * 128, :], ot[:])
                if not last:
                    nc.gpsimd.collective_compute(
                        "ReduceScatter", ALU.add,
                        replica_groups=[list(range(N_CORES))],
                        ins=[bin_[:].opt()], outs=[bout[:].opt()])
                    nc.sync.dma_start(osh[c * 128:(c + 1) * 128, :], bout[:])
    _dedup_ldweights(nc.m)
    nc.compile()
    return nc


def _get_nc():
    if 'nc' not in _CACHE:
        _CACHE['nc'] = _build()
    return _CACHE['nc']


def _make_in_maps(hidden_states, routing_weights, gate_up_proj, gate_up_proj_bias,
                  down_proj, down_proj_bias):
    hs = np.asarray(hidden_states, dtype=np.float32)
    rw = np.asarray(routing_weights, dtype=np.float32)
    gupw = np.asarray(gate_up_proj, dtype=np.float32)
    gupb = np.asarray(gate_up_proj_bias, dtype=np.float32)
    dw = np.asarray(down_proj, dtype=np.float32)
    dbias = np.asarray(down_proj_bias, dtype=np.float32)
    # hsT layout [128, kc*T]: element (h=kc*128+p, t) -> [p, kc*T + t]
    hsT = np.ascontiguousarray(
        hs.T.reshape(KC, 128, T).transpose(1, 0, 2).reshape(128, KC * T).astype(BF16))
    wt_all = np.ascontiguousarray(rw.reshape(T // 128, 128, E).transpose(1, 0, 2))
    in_maps = []
    for e in range(N_CORES):
        g = gupw[e]
        # de-interleave: [gate | up], then tile [128, f_tile*(KC*128) + kc*128 + q]
        gup_de = np.concatenate([g[:, 0::2], g[:, 1::2]], axis=1)  # [H, 2I]
        gup_t = (gup_de.reshape(KC, 128, 2 * NJ, 128).transpose(1, 2, 0, 3)
                 .reshape(128, 2 * NJ * KC * 128).astype(BF16))
        # dwT layout [128, ic*H + h]: element (i=ic*128+p, h) of dw[e].T/1.702
        dwt = ((dw[e].T / np.float32(1.702)).reshape(KC, 128, H)
               .transpose(1, 0, 2).reshape(128, KC * H).astype(BF16))
        in_maps.append({
            "hsT": hsT,
            "gup": np.ascontiguousarray(gup_t),
            # silu(1.702*(x + b)) = silu(1.702*x + 1.702*b); the 1/1.702 glu
            # scale is folded into dwT above.
            "gb": np.ascontiguousarray((1.702 * gupb[e, 0::2]).reshape(NJ, 128).T),
            "ub": np.ascontiguousarray((gupb[e, 1::2] + 1.0).reshape(NJ, 128).T),
            "dwT": np.ascontiguousarray(dwt),
            "db": np.ascontiguousarray(dbias[e][None, :]),
            "wt": np.ascontiguousarray(wt_all[:, :, e]),
        })
    return in_maps


def _assemble(results):
    out = np.empty((T, H), dtype=np.float32)
    for c in range(NCH - 1):
        for r in range(N_CORES):
            out[c * CHUNK + r * 128: c * CHUNK + (r + 1) * 128, :] = \
                results[r]["osh"][c * 128:(c + 1) * 128, :].astype(np.float32)
    tail = np.zeros((CHUNK, H), dtype=np.float32)
    for r in range(N_CORES):
        tail += results[r]["otail"].astype(np.float32)
    out[(NCH - 1) * CHUNK:, :] = tail
    return out


def kernel(hidden_states, routing_weights, gate_up_proj, gate_up_proj_bias,
           down_proj, down_proj_bias):
    from concourse import bass_utils
    in_maps = _make_in_maps(hidden_states, routing_weights, gate_up_proj,
                            gate_up_proj_bias, down_proj, down_proj_bias)
    nc = _get_nc()
    try:
        res = bass_utils.run_bass_kernel_spmd(nc, in_maps, core_ids=list(range(N_CORES)))
    except Exception:
        # One retry in case a previous process left a core wedged.
        res = bass_utils.run_bass_kernel_spmd(nc, in_maps, core_ids=list(range(N_CORES)))
    return _assemble(res.results)
